# revision 2
# baseline (speedup 1.0000x reference)
"""Trainium2 Bass kernel for nn_KANStressPredictor (fp16 planes version).

Per element-triple (s0, s1, s2) of `strain` [B, T, 3]:
    c00 = 2*s0+1, c11 = 2*s1+1, c01 = s2          (C = 2E + I, sym 2x2)
    t1, t2 = eigenvalues of C = (s0+s1+1) -/+ rad, rad = sqrt((s0-s1)^2+s2^2)
    out0, out1 = exp(ki0/3 * (l_i - 0.5*l_other)),  l_i = ln(t_i)
    out2       = ki1 * 0.5 * (l1 + l2)

Layout strategy: host deinterleaves strain into three fp16 planes a, b, c of
shape [128, 8192] per core (pure data-parallel over batch across 8 cores) and
upcasts the fp16 device outputs back to f32. 16-bit planes with unit stride
get the DVE 2x performance mode and halve HBM traffic vs f32.

Engine split per triple:
    DVE:    u = a-b, u2 = u*u, r2 = u2+q, w pair (one stt over the l-block),
            L = l1+l2, o2 = k*L
    GPSIMD: q = c*c
    ACT:    rad = sqrt(r2)   [sqrt table, phase A]
            l block = ln1p(d block), o0/o1 = exp(w block)  [ln/exp table, B]
    PE:     d1 = a+b-rad, d2 = a+b+rad as diagonal-weight matmuls
            accumulated into PSUM (absorbs the s = a+b add)

Phase A runs the sqrt-table work for the whole shard, phase B the ln/exp
work, so the ACT table set switches exactly once inside the NEFF.
"""

import contextlib
import sys

for _p in ("/opt/trn_rl_repo",):
    if _p not in sys.path:
        sys.path.insert(0, _p)

import numpy as np

import concourse.bacc as bacc
import concourse.bass as bass
import concourse.tile as tile
from concourse import mybir
from concourse.bass_utils import run_bass_kernel_spmd

N_CORES = 8
P = 128
F = 8192            # triples per partition per core
CTA = 2048          # phase-A chunk (triples)
CTB = 1024          # phase-B chunk (triples); dd psum tile = [P, 2*CTB] f32

f32 = mybir.dt.float32
f16 = mybir.dt.float16
bf16 = mybir.dt.bfloat16

_cache: dict = {}


def _build(ki0: float, ki1: float, loop_reps: int = 1, use_pe: bool = True,
           use_gpsimd: bool = True):
    key = (ki0, ki1, loop_reps, use_pe, use_gpsimd)
    if key in _cache:
        return _cache[key]

    AF = mybir.ActivationFunctionType
    Add = mybir.AluOpType.add
    Sub = mybir.AluOpType.subtract
    Mult = mybir.AluOpType.mult

    nA = F // CTA
    nB = F // CTB
    rB = CTA // CTB  # B chunks per A chunk

    nc = bacc.Bacc("TRN2", target_bir_lowering=False, debug=False)
    a_ap = nc.dram_tensor("a", [P, F], f16, kind="ExternalInput").ap()
    b_ap = nc.dram_tensor("b", [P, F], f16, kind="ExternalInput").ap()
    c_ap = nc.dram_tensor("c", [P, F], f16, kind="ExternalInput").ap()
    if use_pe:
        # [I | -I] diagonal weights, fp16 for a/b and bf16 for rad operands
        dgh_ap = nc.dram_tensor("dgh", [P, 256], f16, kind="ExternalInput").ap()
        dgb_ap = nc.dram_tensor("dgb", [P, 256], bf16, kind="ExternalInput").ap()
    op_ap = nc.dram_tensor("op", [P, 2 * F], f16, kind="ExternalOutput").ap()
    o2_ap = nc.dram_tensor("o2", [P, F], f16, kind="ExternalOutput").ap()

    with tile.TileContext(nc) as tc:
        with (
            tc.tile_pool(name="persist", bufs=1) as pp,
            tc.tile_pool(name="pa", bufs=2) as pa,
            tc.tile_pool(name="pb", bufs=2) as pb,
            tc.tile_pool(name="io", bufs=3) as iop,
            tc.tile_pool(name="ps", bufs=2, space="PSUM") as psp,
        ):
            if use_pe:
                DGH = pp.tile([P, 256], f16, name="dgh", tag="dgh")
                DGB = pp.tile([P, 256], bf16, name="dgb", tag="dgb")
                nc.sync.dma_start(DGH[:], dgh_ap[:, :])
                nc.sync.dma_start(DGB[:], dgb_ap[:, :])
                Ih = DGH[:][:, 0:128]
                Ib = DGB[:][:, 0:128]
                nIb = DGB[:][:, 128:256]

            # per-A-chunk persistent tiles (distinct tags -> precise deps)
            atk = [pp.tile([P, CTA], f16, name=f"at{i}", tag=f"at{i}")
                   for i in range(nA)]
            btk = [pp.tile([P, CTA], f16, name=f"bt{i}", tag=f"bt{i}")
                   for i in range(nA)]
            radk = [pp.tile([P, CTA], bf16, name=f"rad{i}", tag=f"rad{i}")
                    for i in range(nA)]
            stk = None
            if not use_pe:
                stk = [pp.tile([P, CTA], f16, name=f"st{i}", tag=f"st{i}")
                       for i in range(nA)]

            loop = tc.For_i(0, loop_reps) if loop_reps > 1 else \
                contextlib.nullcontext()
            with loop:
                # ---- Phase A: inputs -> rad (sqrt table) ----
                for ci in range(nA):
                    sl = bass.ts(ci, CTA)
                    AT, BT = atk[ci][:], btk[ci][:]
                    nc.sync.dma_start(AT, a_ap[:, sl])
                    nc.sync.dma_start(BT, b_ap[:, sl])
                    C = pa.tile([P, CTA], f16, name="c", tag="c")
                    nc.sync.dma_start(C[:], c_ap[:, sl])

                    q = pa.tile([P, CTA], bf16, name="q", tag="q")
                    if use_gpsimd:
                        nc.gpsimd.tensor_tensor(q[:], C[:], C[:], Mult)
                    else:
                        nc.vector.tensor_tensor(q[:], C[:], C[:], Mult)
                    u = pa.tile([P, CTA], f16, name="u", tag="u")
                    nc.vector.tensor_tensor(u[:], AT, BT, Sub)
                    if stk is not None:
                        nc.vector.tensor_tensor(stk[ci][:], AT, BT, Add)
                    u2 = pa.tile([P, CTA], bf16, name="u2", tag="u2")
                    nc.vector.tensor_tensor(u2[:], u[:], u[:], Mult)
                    r2 = pa.tile([P, CTA], bf16, name="r2", tag="r2")
                    nc.vector.tensor_tensor(r2[:], u2[:], q[:], Add)
                    nc.scalar.activation(radk[ci][:], r2[:], AF.Sqrt)

                # ---- Phase B: rad -> outputs (ln/exp table) ----
                for cj in range(nB):
                    ca, co = cj // rB, (cj % rB) * CTB
                    AT = atk[ca][:][:, co:co + CTB]
                    BT = btk[ca][:][:, co:co + CTB]
                    RD = radk[ca][:][:, co:co + CTB]

                    if use_pe:
                        dd = psp.tile([P, 2 * CTB], f32, name="dd", tag="dd")
                        DD = dd[:]
                        # d1 in [0:CTB], d2 in [CTB:2*CTB]; 512-wide matmuls
                        for h in range(CTB // 512):
                            hs = slice(h * 512, (h + 1) * 512)
                            d1 = DD[:, h * 512:(h + 1) * 512]
                            d2 = DD[:, CTB + h * 512:CTB + (h + 1) * 512]
                            nc.tensor.matmul(d1, Ih, AT[:, hs], start=True,
                                             stop=False)
                            nc.tensor.matmul(d1, Ih, BT[:, hs], start=False,
                                             stop=False)
                            nc.tensor.matmul(d2, Ih, AT[:, hs], start=True,
                                             stop=False)
                            nc.tensor.matmul(d2, Ih, BT[:, hs], start=False,
                                             stop=False)
                            nc.tensor.matmul(d2, Ib, RD[:, hs], start=False,
                                             stop=True)
                            nc.tensor.matmul(d1, nIb, RD[:, hs], start=False,
                                             stop=True)
                        din = DD
                    else:
                        ST = stk[ca][:][:, co:co + CTB]
                        ddv = pb.tile([P, 2 * CTB], f16, name="ddv", tag="ddv")
                        nc.vector.tensor_tensor(ddv[:][:, 0:CTB], ST, RD, Sub)
                        nc.vector.tensor_tensor(ddv[:][:, CTB:], ST, RD, Add)
                        din = ddv[:]

                    # l block = ln(d + 1): one ACT op over [P, 2*CTB]
                    lt = pb.tile([P, 2 * CTB], f16, name="lt", tag="lt")
                    nc.scalar.activation(lt[:], din, AF.Ln, bias=1.0)
                    l1 = lt[:][:, 0:CTB]
                    l2 = lt[:][:, CTB:]

                    # w block = l - 0.5*lswap: one stt over [P, 2, CTB]
                    W = pb.tile([P, 2 * CTB], f16, name="w", tag="w")
                    ltv = lt[:].rearrange("p (k n) -> p k n", k=2)
                    lsw = ltv[:, ::-1]
                    Wv = W[:].rearrange("p (k n) -> p k n", k=2)
                    nc.vector.scalar_tensor_tensor(Wv, lsw, -0.5, ltv, Mult,
                                                   Add)

                    # o0/o1 = exp(ki0/3 * w), written interleaved
                    OP = iop.tile([P, 2 * CTB], f16, name="opt", tag="opt")
                    OPv = OP[:].rearrange("p (n k) -> p k n", k=2)
                    nc.scalar.activation(OPv, Wv, AF.Exp, scale=ki0 / 3.0)

                    L = pb.tile([P, CTB], f16, name="L", tag="L")
                    nc.vector.tensor_tensor(L[:], l1, l2, Add)
                    O2 = iop.tile([P, CTB], f16, name="o2t", tag="o2t")
                    nc.vector.tensor_scalar_mul(O2[:], L[:], ki1 * 0.5)

                    nc.sync.dma_start(op_ap[:, bass.ts(cj, 2 * CTB)], OP[:])
                    nc.sync.dma_start(o2_ap[:, bass.ts(cj, CTB)], O2[:])

    nc.compile()
    _cache[key] = nc
    return nc


def _diag_weights(dt):
    eye = np.eye(P, dtype=dt)
    return np.concatenate([eye, -eye], axis=1)


def _prep_inputs(strain: np.ndarray):
    """strain [B, T, 3] f32 -> per-core fp16 planes + diag weights."""
    B, T, C = strain.shape
    assert C == 3 and B % N_CORES == 0
    h = strain.astype(np.float16)
    hp = h.reshape(N_CORES, P, F, 3)
    a = np.ascontiguousarray(hp[..., 0])
    b = np.ascontiguousarray(hp[..., 1])
    c = np.ascontiguousarray(hp[..., 2])
    return a, b, c


def _assemble_out(op: np.ndarray, o2: np.ndarray, B: int, T: int):
    """op [8,P,2F] f16 pairs, o2 [8,P,F] f16 -> [B,T,3] f32."""
    out = np.empty((B, T, 3), dtype=np.float32)
    ov = out.reshape(N_CORES, P, F, 3)
    ov[..., 0:2] = op.reshape(N_CORES, P, F, 2)
    ov[..., 2] = o2
    return out


def _run(strain: np.ndarray, ki0: float, ki1: float, trace: bool = False,
         use_pe: bool = True, use_gpsimd: bool = True):
    B, T, C = strain.shape
    a, b, c = _prep_inputs(strain)
    nc = _build(float(ki0), float(ki1), 1, use_pe, use_gpsimd)
    in_maps = []
    for i in range(N_CORES):
        m = {"a": a[i], "b": b[i], "c": c[i]}
        if use_pe:
            m["dgh"] = _diag_weights(np.float16)
            m["dgb"] = _diag_weights(ml_bf16())
        in_maps.append(m)
    res = run_bass_kernel_spmd(nc, in_maps, list(range(N_CORES)), trace=trace)
    op = np.stack([np.asarray(res.results[i]["op"]) for i in range(N_CORES)])
    o2 = np.stack([np.asarray(res.results[i]["o2"]) for i in range(N_CORES)])
    return _assemble_out(op, o2, B, T), res


def ml_bf16():
    import ml_dtypes
    return ml_dtypes.bfloat16


def kernel(strain: np.ndarray, ki0, ki1) -> np.ndarray:
    out, _ = _run(np.asarray(strain), float(np.asarray(ki0)),
                  float(np.asarray(ki1)))
    return out
